# revision 43
# baseline (speedup 1.0000x reference)
"""Trainium2 Bass kernel for AdaptiveModalitySelectionSystem (moe_routing).

Data-parallel over batch B=4096 across 8 NeuronCores (B_local=512 each).

Host-side preprocessing moves all layout/dtype work off the device:
  - x cast to bf16, prepacked to [K, 128, DCH, BL] (partition-major) so each
    DMA descriptor is a 4-8KB contiguous run.
  - W_enc cast to bf16, prepacked to [K, 128, DCH, H].
  - context transposed + packed together with W1 into one tensor CW
    [128, CCH, 576] (cols 0:512 ctx^T chunk, 512:576 W1 chunk) -> one DMA.
  - W2/W3 packed into Q [64, 36]; b1/g_ln/beta_ln/b2/b3p packed into
    P [128, 8] columns; gumbel prepacked [128, NBT, K];
    softmax(fusion_w) folded on host, broadcast-DMAed.
  (dma_start has ~0.7us fixed issue cost per call on the issuing engine, so
   consolidating small DMAs moves the big W stream ~7us earlier.)
Device work per core:
  - Router MLP in transposed layout; LayerNorm stats as PE matmuls against
    an all-1/64 matrix, giving mean/meansq REPLICATED over partitions, so
    var/rsqrt run as [64, 512] ops (no single-partition serial chains).
  - coef[b,k] = mask*(mask>0.5)*softmax(fusion_w)[k] via gumbel-sigmoid +
    forced top-2 (minimax network on logits).
  - Encoder GEMMs: per (k, b-tile, h-block) PSUM accumulation over d.
    k=0 drains as plain copies (independent of the router); coef0 is folded
    into the bias pass acc = acc*coef0 + coefT^T@b_enc; k>=1 drains are
    scalar_tensor_tensor acc = psum*coef_k + acc.
  - Identity-transpose warm-ups keep the PE p-state ramping while the first
    DMAs land; router PE ops are interleaved into the k=0 GEMM stream.
No collectives: each core computes its own output shard independently.
"""
from contextlib import ExitStack

import numpy as np
import ml_dtypes

import concourse.bass as bass
import concourse.tile as tile
from concourse import bacc, mybir
from concourse.bass_utils import run_bass_kernel_spmd
from concourse.masks import make_identity

N_CORES = 8
B, K, D, H, CTX, RH = 4096, 4, 1024, 1024, 256, 64
RH2 = RH // 2
BL = B // N_CORES  # 512 rows per core
NBT = BL // 128    # 4 batch tiles per core
DCH = D // 128     # 8 contraction chunks per modality
CCH = CTX // 128   # 2 contraction chunks for the router
HB = 512           # h-block width (one PSUM bank)
NHB = H // HB      # 2 h-blocks
N_WARM = 12        # PE warm-up transposes
EPS = 1e-5
F32 = mybir.dt.float32
BF16 = mybir.dt.bfloat16
F32R = mybir.dt.float32r
AF = mybir.ActivationFunctionType
OP = mybir.AluOpType
AX = mybir.AxisListType
_BF = ml_dtypes.bfloat16


def _build():
    nc = bacc.Bacc("TRN2", target_bir_lowering=False, debug=False,
                   num_devices=N_CORES)

    def din(name, shape, dt=F32):
        return nc.dram_tensor(name, shape, dt, kind="ExternalInput").ap()

    CW_e = din("CW", [128, CCH, 512 + RH], F32R)
    xP_e = din("xP", [K, 128, DCH, BL], BF16)
    wP_e = din("WP", [K, 128, DCH, H], BF16)
    P_e = din("P", [128, 8])
    Q_e = din("Q", [RH, RH2 + K], F32R)
    gum_e = din("gumP", [128, NBT, K])
    be_e = din("b_encP", [K, H], BF16)
    w4_e = din("w4", [1, K])
    out_e = nc.dram_tensor("out", [BL, H], F32, kind="ExternalOutput").ap()

    with tile.TileContext(nc) as tc, ExitStack() as st:
        singles = st.enter_context(tc.tile_pool(name="singles", bufs=1))
        rt = st.enter_context(tc.tile_pool(name="rt", bufs=2))
        psg = st.enter_context(tc.tile_pool(name="psg", bufs=6, space="PSUM"))
        pst = st.enter_context(tc.tile_pool(name="pst", bufs=2, space="PSUM"))

        # ---- constants ----
        ident = singles.tile([128, 128], F32)
        make_identity(nc, ident[:])
        eps64 = singles.tile([RH, 1], F32)
        nc.vector.memset(eps64[:], EPS)
        inv64_f = singles.tile([RH, RH], F32)
        nc.vector.memset(inv64_f[:], 1.0 / RH)
        inv64 = singles.tile([RH, RH], F32R)
        nc.vector.tensor_copy(out=inv64[:], in_=inv64_f[:])

        # ---- input DMAs ----
        # sync: router pack first, then the W_enc stream, then out (later).
        # gpsimd: the x stream.  scalar: small packs.
        CW = singles.tile([128, CCH, 512 + RH], F32R)
        P = singles.tile([128, 8], F32)
        nc.scalar.dma_start(out=P[:], in_=P_e[:])
        Q = singles.tile([RH, RH2 + K], F32R)
        nc.scalar.dma_start(out=Q[:], in_=Q_e[:])
        gum_sb = singles.tile([128, NBT, K], F32)
        nc.scalar.dma_start(out=gum_sb[:], in_=gum_e[:])
        w4 = singles.tile([128, K], F32)
        nc.scalar.dma_start(out=w4[:], in_=w4_e.to_broadcast([128, K]))
        benc_sb = singles.tile([K, H], BF16)
        nc.scalar.dma_start(out=benc_sb[:], in_=be_e[:])

        b1_c = P[0:RH, 0:1]
        gln_c = P[0:RH, 1:2]
        bln_c = P[0:RH, 2:3]
        b2_c = P[0:RH2, 3:4]
        b3p_c = P[0:K, 4:5]

        acc = singles.tile([128, NBT, H], F32)
        coef = singles.tile([128, NBT, K], F32)
        coefT = singles.tile([K, NBT, 128], BF16)

        xs, ws = [], []
        for k in range(K):
            xst = singles.tile([128, DCH, BL], BF16, name=f"xs{k}")
            wst = singles.tile([128, DCH, H], BF16, name=f"ws{k}")
            xs.append(xst)
            ws.append(wst)

        def xdma(eng, k, cs):
            eng.dma_start(out=xs[k][:, cs, :], in_=xP_e[k, :, cs, :])

        def wdma(eng, k, cs):
            eng.dma_start(out=ws[k][:, cs, :], in_=wP_e[k, :, cs, :])

        # W stream on sync, x stream on gpsimd (mixing W onto the gpsimd
        # queue measured ~25us slower: gpsimd DGE drains serialize behind
        # it; finer-than-quarter k0 granularity measured ~10us slower:
        # dma_start issue overhead compounds).
        wdma(nc.sync, 0, slice(0, 2))
        xdma(nc.gpsimd, 0, slice(0, 2))
        nc.sync.dma_start(out=CW[:], in_=CW_e[:])
        xdma(nc.gpsimd, 0, slice(2, 4))
        wdma(nc.sync, 0, slice(2, 4))
        xdma(nc.gpsimd, 0, slice(4, 6))
        wdma(nc.sync, 0, slice(4, 6))
        xdma(nc.gpsimd, 0, slice(6, 8))
        wdma(nc.sync, 0, slice(6, 8))
        for k in range(1, K):
            wdma(nc.sync, k, slice(0, 4))
            xdma(nc.gpsimd, k, slice(0, 4))
            wdma(nc.sync, k, slice(4, 8))
            xdma(nc.gpsimd, k, slice(4, 8))

        # ---- PE warm-up: keeps the p-state ramp going while DMAs land ----
        for i in range(N_WARM):
            wps = pst.tile([128, 128], F32, tag="ps", name=f"warm{i}")
            nc.tensor.transpose(out=wps[:], in_=ident[:], identity=ident[:])

        # ---- encoder GEMM block for one (k, bt): 16 matmuls (+ drain) ----
        k0_pms = {}

        def emit_bt(k, bt):
            pms = [psg.tile([128, HB], F32, tag="pm", name=f"pm{k}_{bt}_{hb}")
                   for hb in range(NHB)]
            for c in range(DCH):
                for hb in range(NHB):
                    nc.tensor.matmul(out=pms[hb][:],
                                     lhsT=xs[k][:, c, bt * 128:(bt + 1) * 128],
                                     rhs=ws[k][:, c, hb * HB:(hb + 1) * HB],
                                     start=(c == 0), stop=(c == DCH - 1))
            if k == 0:
                # coef not ready yet: drains deferred (Scalar-engine copies
                # emitted later); coef0 folded into the bias pass below.
                k0_pms[bt] = pms
                return
            for hb in range(NHB):
                hsl = slice(hb * HB, (hb + 1) * HB)
                nc.vector.scalar_tensor_tensor(out=acc[:, bt, hsl],
                                               in0=pms[hb][:],
                                               scalar=coef[:, bt, k:k + 1],
                                               in1=acc[:, bt, hsl],
                                               op0=OP.mult, op1=OP.add)
                if k == K - 1:
                    # per-hb writes let the h0 half ship while h1 drains
                    nc.sync.dma_start(
                        out=out_e[bt * 128:(bt + 1) * 128, hsl],
                        in_=acc[:, bt, hsl])

        def drain_k0(bt):
            # plain PSUM->SBUF copies on the (idle) Scalar engine, keeping
            # the Vector queue free for the router chain
            for hb in range(NHB):
                hsl = slice(hb * HB, (hb + 1) * HB)
                nc.scalar.activation(out=acc[:, bt, hsl],
                                     in_=k0_pms[bt][hb][:], func=AF.Copy)

        emit_bt(0, 0)

        # ---- router part 1: hT = (ctx @ W1 + b1)^T, LN stat matmuls ----
        hps = pst.tile([RH, BL], F32, tag="ps")
        nc.tensor.matmul(out=hps[:], lhsT=CW[:, 0, 512:512 + RH],
                         rhs=CW[:, 0, 0:512], start=True, stop=False)
        nc.tensor.matmul(out=hps[:], lhsT=CW[:, 1, 512:512 + RH],
                         rhs=CW[:, 1, 0:512], start=False, stop=True)
        hT_raw = rt.tile([RH, BL], F32R, tag="hT_raw")
        nc.vector.tensor_scalar_add(out=hT_raw[:], in0=hps[:], scalar1=b1_c)
        hsq = rt.tile([RH, BL], F32R, tag="hsq")
        nc.vector.tensor_tensor(out=hsq[:], in0=hT_raw[:], in1=hT_raw[:],
                                op=OP.mult)
        # mean / mean-square REPLICATED over all 64 partitions via an
        # all-1/64 stationary matrix -> no [1, BL] single-lane DVE ops.
        mups = pst.tile([RH, BL], F32, tag="ps", name="mups")
        nc.tensor.matmul(out=mups[:], lhsT=inv64[:], rhs=hT_raw[:],
                         start=True, stop=True)
        msps = pst.tile([RH, BL], F32, tag="ps", name="msps")
        nc.tensor.matmul(out=msps[:], lhsT=inv64[:], rhs=hsq[:],
                         start=True, stop=True)

        emit_bt(0, 1)

        # ---- router part 2: var, rstd, normalize, GEMM2/3, logits^T ----
        # (PSUM tiles may appear at most once per DVE op, so mean goes
        # through an SBUF copy first)
        mu_sb = rt.tile([RH, BL], F32, tag="mu_sb")
        nc.vector.tensor_copy(out=mu_sb[:], in_=mups[:])
        musq = rt.tile([RH, BL], F32, tag="musq")
        nc.vector.tensor_tensor(out=musq[:], in0=mu_sb[:], in1=mu_sb[:],
                                op=OP.mult)
        var_bc = rt.tile([RH, BL], F32, tag="var_bc")
        nc.vector.tensor_tensor(out=var_bc[:], in0=msps[:], in1=musq[:],
                                op=OP.subtract)
        # rstd = exp(-0.5*ln(var+eps)); DVE has no divide and Rsqrt is
        # blocked for accuracy, so route through the ACT tables.
        lnv = rt.tile([RH, BL], F32, tag="lnv")
        nc.scalar.activation(out=lnv[:], in_=var_bc[:], func=AF.Ln,
                             bias=eps64[:])
        rstd_bc = rt.tile([RH, BL], F32, tag="rstd_bc")
        nc.scalar.activation(out=rstd_bc[:], in_=lnv[:], func=AF.Exp,
                             scale=-0.5)
        hn = rt.tile([RH, BL], F32R, tag="hn")
        nc.vector.tensor_tensor(out=hn[:], in0=hT_raw[:], in1=mu_sb[:],
                                op=OP.subtract)
        nc.vector.tensor_tensor(out=hn[:], in0=hn[:], in1=rstd_bc[:],
                                op=OP.mult)
        nc.vector.tensor_scalar(out=hn[:], in0=hn[:], scalar1=gln_c,
                                scalar2=bln_c, op0=OP.mult, op1=OP.add)
        nc.vector.tensor_single_scalar(out=hn[:], in_=hn[:], scalar=0.0,
                                       op=OP.max)

        emit_bt(0, 2)

        ps3 = pst.tile([RH2, BL], F32, tag="ps")
        nc.tensor.matmul(out=ps3[:], lhsT=Q[:, 0:RH2], rhs=hn[:],
                         start=True, stop=True)
        h2T = rt.tile([RH2, BL], F32R, tag="h2T")
        nc.vector.tensor_scalar(out=h2T[:], in0=ps3[:], scalar1=b2_c,
                                scalar2=0.0, op0=OP.add, op1=OP.max)
        ps4 = pst.tile([K, BL], F32, tag="ps")
        nc.tensor.matmul(out=ps4[:], lhsT=Q[0:RH2, RH2:RH2 + K], rhs=h2T[:],
                         start=True, stop=True)
        lgT = rt.tile([K, BL], F32, tag="lgT")
        nc.vector.tensor_scalar_add(out=lgT[:], in0=ps4[:], scalar1=b3p_c)

        emit_bt(0, 3)
        drain_k0(0)
        drain_k0(1)

        # ---- router part 3: logits to [b, K], mask pipeline, coef ----
        lg = singles.tile([128, NBT, K], F32)
        for bt in range(NBT):
            ps5 = pst.tile([128, K], F32, tag="ps", name=f"ps5_{bt}")
            nc.tensor.transpose(out=ps5[:], in_=lgT[:, bt * 128:(bt + 1) * 128],
                                identity=ident[0:K, 0:K])
            nc.vector.tensor_copy(out=lg[:, bt, :], in_=ps5[:])

        s_all = rt.tile([128, NBT, K], F32, tag="s_all")
        nc.vector.tensor_tensor(out=s_all[:], in0=lg[:], in1=gum_sb[:], op=OP.add)
        soft_all = rt.tile([128, NBT, K], F32, tag="soft_all")
        nc.scalar.activation(out=soft_all[:], in_=s_all[:], func=AF.Sigmoid)

        # top-2 of 4 via minimax network (on logits; sigmoid is monotonic)
        a, b = lg[:, :, 0:1], lg[:, :, 1:2]
        c_, d_ = lg[:, :, 2:3], lg[:, :, 3:4]
        mab = rt.tile([128, NBT, 1], F32, tag="mab")
        nc.vector.tensor_tensor(out=mab[:], in0=a, in1=b, op=OP.max)
        mcd = rt.tile([128, NBT, 1], F32, tag="mcd")
        nc.vector.tensor_tensor(out=mcd[:], in0=c_, in1=d_, op=OP.max)
        nab = rt.tile([128, NBT, 1], F32, tag="nab")
        nc.vector.tensor_tensor(out=nab[:], in0=a, in1=b, op=OP.min)
        ncd = rt.tile([128, NBT, 1], F32, tag="ncd")
        nc.vector.tensor_tensor(out=ncd[:], in0=c_, in1=d_, op=OP.min)
        mmm = rt.tile([128, NBT, 1], F32, tag="mmm")
        nc.vector.tensor_tensor(out=mmm[:], in0=mab[:], in1=mcd[:], op=OP.min)
        m2a = rt.tile([128, NBT, 1], F32, tag="m2a")
        nc.vector.tensor_tensor(out=m2a[:], in0=nab[:], in1=ncd[:], op=OP.max)
        m2b = rt.tile([128, NBT, 1], F32, tag="m2b")
        nc.vector.tensor_tensor(out=m2b[:], in0=m2a[:], in1=mmm[:], op=OP.max)

        mnm = rt.tile([128, NBT, K], F32, tag="mnm")
        for kk in range(K):
            nc.vector.tensor_tensor(out=mnm[:, :, kk:kk + 1],
                                    in0=lg[:, :, kk:kk + 1],
                                    in1=m2b[:], op=OP.is_ge)
        msk = rt.tile([128, NBT, K], F32, tag="msk")
        nc.vector.tensor_tensor(out=msk[:], in0=soft_all[:], in1=mnm[:], op=OP.max)
        hm = rt.tile([128, NBT, K], F32, tag="hm")
        nc.vector.scalar_tensor_tensor(out=hm[:], in0=msk[:], scalar=0.5,
                                       in1=msk[:], op0=OP.is_gt, op1=OP.mult)
        for kk in range(K):
            nc.vector.tensor_scalar_mul(out=coef[:, :, kk:kk + 1],
                                        in0=hm[:, :, kk:kk + 1],
                                        scalar1=w4[:, kk:kk + 1])

        drain_k0(2)
        drain_k0(3)

        # ---- bias + coef0 fold: acc = acc*coef0 + coefT^T @ b_enc ----
        # coefT up-front (tiny), bias matmuls interleaved into the k=1
        # stream so the bias PSUM-slot recycling (gated on Vector drains)
        # overlaps GEMM instead of stalling the PE
        for bt in range(NBT):
            ps6 = pst.tile([K, 128], F32, tag="ps", name=f"ps6_{bt}")
            nc.tensor.transpose(out=ps6[:], in_=coef[:, bt, :], identity=ident[:])
            nc.vector.tensor_copy(out=coefT[:, bt, :], in_=ps6[:])

        def emit_bias(bt):
            for hb in range(NHB):
                hsl = slice(hb * HB, (hb + 1) * HB)
                pmb = pst.tile([128, HB], F32, tag="ps", name=f"pmb{bt}_{hb}")
                nc.tensor.matmul(out=pmb[:], lhsT=coefT[:, bt, :],
                                 rhs=benc_sb[:, hsl], start=True, stop=True)
                nc.vector.scalar_tensor_tensor(out=acc[:, bt, hsl],
                                               in0=acc[:, bt, hsl],
                                               scalar=coef[:, bt, 0:1],
                                               in1=pmb[:],
                                               op0=OP.mult, op1=OP.add)

        for bt in range(NBT):
            emit_bias(bt)
            emit_bt(1, bt)

        def emit_k_pipelined(k):
            # lag-4 software pipeline: consecutive accumulation groups
            # overlap, so each group's start/stop (a ~216ns PE pipeline
            # bubble at the boundary) lands mid-stream of its neighbour
            groups = [(bt, hb) for bt in range(NBT) for hb in range(NHB)]
            pms = {g: psg.tile([128, HB], F32, tag="pm",
                               name=f"pm{k}_{g[0]}_{g[1]}") for g in groups}
            sched = []
            prev = None
            for g in groups:
                if prev is None:
                    for c in range(DCH // 2):
                        sched.append((g, c))
                else:
                    for c in range(DCH // 2):
                        sched.append((prev, DCH // 2 + c))
                        sched.append((g, c))
                prev = g
            for c in range(DCH // 2, DCH):
                sched.append((prev, c))
            for g, c in sched:
                bt, hb = g
                nc.tensor.matmul(out=pms[g][:],
                                 lhsT=xs[k][:, c, bt * 128:(bt + 1) * 128],
                                 rhs=ws[k][:, c, hb * HB:(hb + 1) * HB],
                                 start=(c == 0), stop=(c == DCH - 1))
                if c == DCH - 1:
                    hsl = slice(hb * HB, (hb + 1) * HB)
                    nc.vector.scalar_tensor_tensor(out=acc[:, bt, hsl],
                                                   in0=pms[g][:],
                                                   scalar=coef[:, bt, k:k + 1],
                                                   in1=acc[:, bt, hsl],
                                                   op0=OP.mult, op1=OP.add)
                    if k == K - 1:
                        nc.sync.dma_start(
                            out=out_e[bt * 128:(bt + 1) * 128, hsl],
                            in_=acc[:, bt, hsl])

        emit_k_pipelined(2)
        emit_k_pipelined(3)

    nc.compile()
    return nc


_NC = None


def _get_nc():
    global _NC
    if _NC is None:
        _NC = _build()
    return _NC


def _softmax(v):
    e = np.exp(v - np.max(v))
    return e / e.sum()


def _make_in_maps(inputs):
    f = {k: np.asarray(v) for k, v in inputs.items()}
    x_bf = f["x"].astype(_BF)                       # [K, B, D]
    W1P = (f["W1"].astype(np.float32).reshape(CCH, 128, RH)
           .transpose(1, 0, 2))                     # [128, CCH, RH]
    WP = np.ascontiguousarray(
        f["W_enc"].astype(_BF).reshape(K, DCH, 128, H).transpose(0, 2, 1, 3))
    w4 = _softmax(f["fusion_w"].astype(np.float64).ravel()).astype(np.float32)
    b3p = (f["b3"].astype(np.float32) + f["prior"].astype(np.float32)).ravel()
    P = np.zeros((128, 8), dtype=np.float32)
    P[0:RH, 0] = f["b1"].astype(np.float32).ravel()
    P[0:RH, 1] = f["g_ln"].astype(np.float32).ravel()
    P[0:RH, 2] = f["beta_ln"].astype(np.float32).ravel()
    P[0:RH2, 3] = f["b2"].astype(np.float32).ravel()
    P[0:K, 4] = b3p
    Q = np.zeros((RH, RH2 + K), dtype=np.float32)
    Q[:, 0:RH2] = f["W2"].astype(np.float32)
    Q[0:RH2, RH2:RH2 + K] = f["W3"].astype(np.float32)
    shared = {
        "P": P,
        "Q": Q,
        "WP": WP,
        "b_encP": np.ascontiguousarray(f["b_enc"].astype(_BF)),
        "w4": w4.reshape(1, K),
    }
    in_maps = []
    for i in range(N_CORES):
        sl = slice(i * BL, (i + 1) * BL)
        m = dict(shared)
        # ctxP[p, c, b] = context[b, c*128+p]; packed with W1P -> CW
        ctxP = (f["context"][sl].astype(np.float32).T.reshape(CCH, 128, BL)
                .transpose(1, 0, 2))
        m["CW"] = np.ascontiguousarray(
            np.concatenate([ctxP, W1P], axis=2))
        # xP[k, p, c, b] = x[k, b, c*128+p]
        m["xP"] = np.ascontiguousarray(
            x_bf[:, sl, :].transpose(0, 2, 1).reshape(K, DCH, 128, BL)
            .transpose(0, 2, 1, 3))
        # gumP[p, t, k] = gumbel[t*128+p, k]
        m["gumP"] = np.ascontiguousarray(
            f["gumbel"][sl].astype(np.float32).reshape(NBT, 128, K)
            .transpose(1, 0, 2))
        in_maps.append(m)
    return in_maps


def kernel(**inputs):
    nc = _get_nc()
    in_maps = _make_in_maps(inputs)
    res = run_bass_kernel_spmd(nc, in_maps, core_ids=list(range(N_CORES)))
    return np.concatenate([res.results[i]["out"] for i in range(N_CORES)],
                          axis=0)


# revision 45
# speedup vs baseline: 1.1518x; 1.1518x over previous
"""Trainium2 Bass kernel for AdaptiveModalitySelectionSystem (moe_routing).

Data-parallel over batch B=4096 across 8 NeuronCores (B_local=512 each).

Host-side preprocessing moves all layout/dtype work off the device:
  - x cast to bf16, prepacked to [K, 128, DCH, BL] (partition-major) so each
    DMA descriptor is a 4-8KB contiguous run.
  - W_enc cast to bf16, prepacked to [K, 128, DCH, H].
  - context transposed + packed together with W1 into one tensor CW
    [128, CCH, 576] (cols 0:512 ctx^T chunk, 512:576 W1 chunk) -> one DMA.
  - W2/W3 packed into Q [64, 36]; b1/g_ln/beta_ln/b2/b3p packed into
    P [128, 8] columns; gumbel prepacked [128, NBT, K];
    softmax(fusion_w) folded on host, broadcast-DMAed.
  (dma_start has ~0.7us fixed issue cost per call on the issuing engine, so
   consolidating small DMAs moves the big W stream ~7us earlier.)
Device work per core:
  - Router MLP in transposed layout; LayerNorm stats as PE matmuls against
    an all-1/64 matrix, giving mean/meansq REPLICATED over partitions, so
    var/rsqrt run as [64, 512] ops (no single-partition serial chains).
  - coef[b,k] = mask*(mask>0.5)*softmax(fusion_w)[k] via gumbel-sigmoid +
    forced top-2 (minimax network on logits).
  - Encoder GEMMs: per (k, b-tile, h-block) PSUM accumulation over d.
    k=0 drains as plain copies (independent of the router); coef0 is folded
    into the bias pass acc = acc*coef0 + coefT^T@b_enc; k>=1 drains are
    scalar_tensor_tensor acc = psum*coef_k + acc.
  - Identity-transpose warm-ups keep the PE p-state ramping while the first
    DMAs land; router PE ops are interleaved into the k=0 GEMM stream.
No collectives: each core computes its own output shard independently.
"""
from contextlib import ExitStack

import numpy as np
import ml_dtypes

import concourse.bass as bass
import concourse.tile as tile
from concourse import bacc, mybir
from concourse.bass_utils import run_bass_kernel_spmd
from concourse.masks import make_identity

N_CORES = 8
B, K, D, H, CTX, RH = 4096, 4, 1024, 1024, 256, 64
RH2 = RH // 2
BL = B // N_CORES  # 512 rows per core
NBT = BL // 128    # 4 batch tiles per core
DCH = D // 128     # 8 contraction chunks per modality
CCH = CTX // 128   # 2 contraction chunks for the router
HB = 512           # h-block width (one PSUM bank)
NHB = H // HB      # 2 h-blocks
N_WARM = 12        # PE warm-up transposes
EPS = 1e-5
F32 = mybir.dt.float32
BF16 = mybir.dt.bfloat16
F32R = mybir.dt.float32r
AF = mybir.ActivationFunctionType
OP = mybir.AluOpType
AX = mybir.AxisListType
_BF = ml_dtypes.bfloat16


def _build():
    nc = bacc.Bacc("TRN2", target_bir_lowering=False, debug=False,
                   num_devices=N_CORES)

    def din(name, shape, dt=F32):
        return nc.dram_tensor(name, shape, dt, kind="ExternalInput").ap()

    CW_e = din("CW", [128, CCH, 512 + RH], F32R)
    xP_e = din("xP", [K, 128, DCH, BL], BF16)
    wP_e = din("WP", [K, 128, DCH, H], BF16)
    P_e = din("P", [128, 8])
    Q_e = din("Q", [RH, RH2 + K], F32R)
    gum_e = din("gumP", [128, NBT, K])
    be_e = din("b_encP", [K, H], BF16)
    w4_e = din("w4", [1, K])
    out_e = nc.dram_tensor("out", [BL, H], F32, kind="ExternalOutput").ap()

    with tile.TileContext(nc) as tc, ExitStack() as st:
        singles = st.enter_context(tc.tile_pool(name="singles", bufs=1))
        rt = st.enter_context(tc.tile_pool(name="rt", bufs=2))
        psg = st.enter_context(tc.tile_pool(name="psg", bufs=6, space="PSUM"))
        pst = st.enter_context(tc.tile_pool(name="pst", bufs=2, space="PSUM"))

        # ---- constants ----
        ident = singles.tile([128, 128], F32)
        make_identity(nc, ident[:])
        eps64 = singles.tile([RH, 1], F32)
        nc.vector.memset(eps64[:], EPS)
        inv64_f = singles.tile([RH, RH], F32)
        nc.vector.memset(inv64_f[:], 1.0 / RH)
        inv64 = singles.tile([RH, RH], F32R)
        nc.vector.tensor_copy(out=inv64[:], in_=inv64_f[:])

        # ---- input DMAs ----
        # sync: router pack first, then the W_enc stream, then out (later).
        # gpsimd: the x stream.  scalar: small packs.
        CW = singles.tile([128, CCH, 512 + RH], F32R)
        P = singles.tile([128, 8], F32)
        nc.scalar.dma_start(out=P[:], in_=P_e[:])
        Q = singles.tile([RH, RH2 + K], F32R)
        nc.scalar.dma_start(out=Q[:], in_=Q_e[:])
        gum_sb = singles.tile([128, NBT, K], F32)
        nc.scalar.dma_start(out=gum_sb[:], in_=gum_e[:])
        w4 = singles.tile([128, K], F32)
        nc.scalar.dma_start(out=w4[:], in_=w4_e.to_broadcast([128, K]))
        benc_sb = singles.tile([K, H], BF16)
        nc.scalar.dma_start(out=benc_sb[:], in_=be_e[:])

        b1_c = P[0:RH, 0:1]
        gln_c = P[0:RH, 1:2]
        bln_c = P[0:RH, 2:3]
        b2_c = P[0:RH2, 3:4]
        b3p_c = P[0:K, 4:5]

        acc = singles.tile([128, NBT, H], F32)
        coef = singles.tile([128, NBT, K], F32)
        coefT = singles.tile([K, NBT, 128], BF16)

        xs, ws = [], []
        for k in range(K):
            xst = singles.tile([128, DCH, BL], BF16, name=f"xs{k}")
            wst = singles.tile([128, DCH, H], BF16, name=f"ws{k}")
            xs.append(xst)
            ws.append(wst)

        def xdma(eng, k, cs):
            eng.dma_start(out=xs[k][:, cs, :], in_=xP_e[k, :, cs, :])

        def wdma(eng, k, cs):
            eng.dma_start(out=ws[k][:, cs, :], in_=wP_e[k, :, cs, :])

        # W stream on sync, x stream on gpsimd (mixing W onto the gpsimd
        # queue measured ~25us slower: gpsimd DGE drains serialize behind
        # it; finer-than-quarter k0 granularity measured ~10us slower:
        # dma_start issue overhead compounds).
        wdma(nc.sync, 0, slice(0, 2))
        xdma(nc.gpsimd, 0, slice(0, 2))
        xdma(nc.gpsimd, 0, slice(2, 4))
        wdma(nc.sync, 0, slice(2, 4))
        # CW (router pack) after W0's first half: k0-bt0 is gated by its
        # last W0 chunk, while the router has ~2us of slack before coef is
        # consumed -> shipping W0 first ends the data-bound k0 phase earlier
        nc.sync.dma_start(out=CW[:], in_=CW_e[:])
        xdma(nc.gpsimd, 0, slice(4, 6))
        wdma(nc.sync, 0, slice(4, 6))
        xdma(nc.gpsimd, 0, slice(6, 8))
        wdma(nc.sync, 0, slice(6, 8))
        for k in range(1, K):
            wdma(nc.sync, k, slice(0, 4))
            xdma(nc.gpsimd, k, slice(0, 4))
            wdma(nc.sync, k, slice(4, 8))
            xdma(nc.gpsimd, k, slice(4, 8))

        # ---- PE warm-up: keeps the p-state ramp going while DMAs land ----
        for i in range(N_WARM):
            wps = pst.tile([128, 128], F32, tag="ps", name=f"warm{i}")
            nc.tensor.transpose(out=wps[:], in_=ident[:], identity=ident[:])

        # ---- encoder GEMM block for one (k, bt): 16 matmuls (+ drain) ----
        k0_pms = {}

        def emit_bt(k, bt):
            pms = [psg.tile([128, HB], F32, tag="pm", name=f"pm{k}_{bt}_{hb}")
                   for hb in range(NHB)]
            for c in range(DCH):
                for hb in range(NHB):
                    nc.tensor.matmul(out=pms[hb][:],
                                     lhsT=xs[k][:, c, bt * 128:(bt + 1) * 128],
                                     rhs=ws[k][:, c, hb * HB:(hb + 1) * HB],
                                     start=(c == 0), stop=(c == DCH - 1))
            if k == 0:
                # coef not ready yet: drains deferred (Scalar-engine copies
                # emitted later); coef0 folded into the bias pass below.
                k0_pms[bt] = pms
                return
            for hb in range(NHB):
                hsl = slice(hb * HB, (hb + 1) * HB)
                nc.vector.scalar_tensor_tensor(out=acc[:, bt, hsl],
                                               in0=pms[hb][:],
                                               scalar=coef[:, bt, k:k + 1],
                                               in1=acc[:, bt, hsl],
                                               op0=OP.mult, op1=OP.add)
                if k == K - 1:
                    # per-hb writes let the h0 half ship while h1 drains
                    nc.sync.dma_start(
                        out=out_e[bt * 128:(bt + 1) * 128, hsl],
                        in_=acc[:, bt, hsl])

        def drain_k0(bt):
            # plain PSUM->SBUF copies on the (idle) Scalar engine, keeping
            # the Vector queue free for the router chain
            for hb in range(NHB):
                hsl = slice(hb * HB, (hb + 1) * HB)
                nc.scalar.activation(out=acc[:, bt, hsl],
                                     in_=k0_pms[bt][hb][:], func=AF.Copy)

        emit_bt(0, 0)

        # ---- router part 1: hT = (ctx @ W1 + b1)^T, LN stat matmuls ----
        hps = pst.tile([RH, BL], F32, tag="ps")
        nc.tensor.matmul(out=hps[:], lhsT=CW[:, 0, 512:512 + RH],
                         rhs=CW[:, 0, 0:512], start=True, stop=False)
        nc.tensor.matmul(out=hps[:], lhsT=CW[:, 1, 512:512 + RH],
                         rhs=CW[:, 1, 0:512], start=False, stop=True)
        hT_raw = rt.tile([RH, BL], F32R, tag="hT_raw")
        nc.vector.tensor_scalar_add(out=hT_raw[:], in0=hps[:], scalar1=b1_c)
        hsq = rt.tile([RH, BL], F32R, tag="hsq")
        nc.vector.tensor_tensor(out=hsq[:], in0=hT_raw[:], in1=hT_raw[:],
                                op=OP.mult)
        # mean / mean-square REPLICATED over all 64 partitions via an
        # all-1/64 stationary matrix -> no [1, BL] single-lane DVE ops.
        mups = pst.tile([RH, BL], F32, tag="ps", name="mups")
        nc.tensor.matmul(out=mups[:], lhsT=inv64[:], rhs=hT_raw[:],
                         start=True, stop=True)
        msps = pst.tile([RH, BL], F32, tag="ps", name="msps")
        nc.tensor.matmul(out=msps[:], lhsT=inv64[:], rhs=hsq[:],
                         start=True, stop=True)

        emit_bt(0, 1)

        # ---- router part 2: var, rstd, normalize, GEMM2/3, logits^T ----
        # (PSUM tiles may appear at most once per DVE op, so mean goes
        # through an SBUF copy first)
        mu_sb = rt.tile([RH, BL], F32, tag="mu_sb")
        nc.vector.tensor_copy(out=mu_sb[:], in_=mups[:])
        musq = rt.tile([RH, BL], F32, tag="musq")
        nc.vector.tensor_tensor(out=musq[:], in0=mu_sb[:], in1=mu_sb[:],
                                op=OP.mult)
        var_bc = rt.tile([RH, BL], F32, tag="var_bc")
        nc.vector.tensor_tensor(out=var_bc[:], in0=msps[:], in1=musq[:],
                                op=OP.subtract)
        # rstd = exp(-0.5*ln(var+eps)); DVE has no divide and Rsqrt is
        # blocked for accuracy, so route through the ACT tables.
        lnv = rt.tile([RH, BL], F32, tag="lnv")
        nc.scalar.activation(out=lnv[:], in_=var_bc[:], func=AF.Ln,
                             bias=eps64[:])
        rstd_bc = rt.tile([RH, BL], F32, tag="rstd_bc")
        nc.scalar.activation(out=rstd_bc[:], in_=lnv[:], func=AF.Exp,
                             scale=-0.5)
        hn = rt.tile([RH, BL], F32R, tag="hn")
        nc.vector.tensor_tensor(out=hn[:], in0=hT_raw[:], in1=mu_sb[:],
                                op=OP.subtract)
        nc.vector.tensor_tensor(out=hn[:], in0=hn[:], in1=rstd_bc[:],
                                op=OP.mult)
        nc.vector.tensor_scalar(out=hn[:], in0=hn[:], scalar1=gln_c,
                                scalar2=bln_c, op0=OP.mult, op1=OP.add)
        nc.vector.tensor_single_scalar(out=hn[:], in_=hn[:], scalar=0.0,
                                       op=OP.max)

        emit_bt(0, 2)

        ps3 = pst.tile([RH2, BL], F32, tag="ps")
        nc.tensor.matmul(out=ps3[:], lhsT=Q[:, 0:RH2], rhs=hn[:],
                         start=True, stop=True)
        h2T = rt.tile([RH2, BL], F32R, tag="h2T")
        nc.vector.tensor_scalar(out=h2T[:], in0=ps3[:], scalar1=b2_c,
                                scalar2=0.0, op0=OP.add, op1=OP.max)
        ps4 = pst.tile([K, BL], F32, tag="ps")
        nc.tensor.matmul(out=ps4[:], lhsT=Q[0:RH2, RH2:RH2 + K], rhs=h2T[:],
                         start=True, stop=True)
        lgT = rt.tile([K, BL], F32, tag="lgT")
        nc.vector.tensor_scalar_add(out=lgT[:], in0=ps4[:], scalar1=b3p_c)

        emit_bt(0, 3)
        drain_k0(0)
        drain_k0(1)

        # ---- router part 3: logits to [b, K], mask pipeline, coef ----
        lg = singles.tile([128, NBT, K], F32)
        for bt in range(NBT):
            ps5 = pst.tile([128, K], F32, tag="ps", name=f"ps5_{bt}")
            nc.tensor.transpose(out=ps5[:], in_=lgT[:, bt * 128:(bt + 1) * 128],
                                identity=ident[0:K, 0:K])
            nc.vector.tensor_copy(out=lg[:, bt, :], in_=ps5[:])

        s_all = rt.tile([128, NBT, K], F32, tag="s_all")
        nc.vector.tensor_tensor(out=s_all[:], in0=lg[:], in1=gum_sb[:], op=OP.add)
        soft_all = rt.tile([128, NBT, K], F32, tag="soft_all")
        nc.scalar.activation(out=soft_all[:], in_=s_all[:], func=AF.Sigmoid)

        # top-2 of 4 via minimax network (on logits; sigmoid is monotonic)
        a, b = lg[:, :, 0:1], lg[:, :, 1:2]
        c_, d_ = lg[:, :, 2:3], lg[:, :, 3:4]
        mab = rt.tile([128, NBT, 1], F32, tag="mab")
        nc.vector.tensor_tensor(out=mab[:], in0=a, in1=b, op=OP.max)
        mcd = rt.tile([128, NBT, 1], F32, tag="mcd")
        nc.vector.tensor_tensor(out=mcd[:], in0=c_, in1=d_, op=OP.max)
        nab = rt.tile([128, NBT, 1], F32, tag="nab")
        nc.vector.tensor_tensor(out=nab[:], in0=a, in1=b, op=OP.min)
        ncd = rt.tile([128, NBT, 1], F32, tag="ncd")
        nc.vector.tensor_tensor(out=ncd[:], in0=c_, in1=d_, op=OP.min)
        mmm = rt.tile([128, NBT, 1], F32, tag="mmm")
        nc.vector.tensor_tensor(out=mmm[:], in0=mab[:], in1=mcd[:], op=OP.min)
        m2a = rt.tile([128, NBT, 1], F32, tag="m2a")
        nc.vector.tensor_tensor(out=m2a[:], in0=nab[:], in1=ncd[:], op=OP.max)
        m2b = rt.tile([128, NBT, 1], F32, tag="m2b")
        nc.vector.tensor_tensor(out=m2b[:], in0=m2a[:], in1=mmm[:], op=OP.max)

        mnm = rt.tile([128, NBT, K], F32, tag="mnm")
        for kk in range(K):
            nc.vector.tensor_tensor(out=mnm[:, :, kk:kk + 1],
                                    in0=lg[:, :, kk:kk + 1],
                                    in1=m2b[:], op=OP.is_ge)
        msk = rt.tile([128, NBT, K], F32, tag="msk")
        nc.vector.tensor_tensor(out=msk[:], in0=soft_all[:], in1=mnm[:], op=OP.max)
        hm = rt.tile([128, NBT, K], F32, tag="hm")
        nc.vector.scalar_tensor_tensor(out=hm[:], in0=msk[:], scalar=0.5,
                                       in1=msk[:], op0=OP.is_gt, op1=OP.mult)
        for kk in range(K):
            nc.vector.tensor_scalar_mul(out=coef[:, :, kk:kk + 1],
                                        in0=hm[:, :, kk:kk + 1],
                                        scalar1=w4[:, kk:kk + 1])

        drain_k0(2)
        drain_k0(3)

        # ---- bias + coef0 fold: acc = acc*coef0 + coefT^T @ b_enc ----
        # coefT up-front (tiny), bias matmuls interleaved into the k=1
        # stream so the bias PSUM-slot recycling (gated on Vector drains)
        # overlaps GEMM instead of stalling the PE
        for bt in range(NBT):
            ps6 = pst.tile([K, 128], F32, tag="ps", name=f"ps6_{bt}")
            nc.tensor.transpose(out=ps6[:], in_=coef[:, bt, :], identity=ident[:])
            nc.vector.tensor_copy(out=coefT[:, bt, :], in_=ps6[:])

        def emit_bias(bt):
            for hb in range(NHB):
                hsl = slice(hb * HB, (hb + 1) * HB)
                pmb = pst.tile([128, HB], F32, tag="ps", name=f"pmb{bt}_{hb}")
                nc.tensor.matmul(out=pmb[:], lhsT=coefT[:, bt, :],
                                 rhs=benc_sb[:, hsl], start=True, stop=True)
                nc.vector.scalar_tensor_tensor(out=acc[:, bt, hsl],
                                               in0=acc[:, bt, hsl],
                                               scalar=coef[:, bt, 0:1],
                                               in1=pmb[:],
                                               op0=OP.mult, op1=OP.add)

        for bt in range(NBT):
            emit_bias(bt)
            emit_bt(1, bt)
        for k in range(2, K):
            for bt in range(NBT):
                emit_bt(k, bt)

    nc.compile()
    return nc


_NC = None


def _get_nc():
    global _NC
    if _NC is None:
        _NC = _build()
    return _NC


def _softmax(v):
    e = np.exp(v - np.max(v))
    return e / e.sum()


def _make_in_maps(inputs):
    f = {k: np.asarray(v) for k, v in inputs.items()}
    x_bf = f["x"].astype(_BF)                       # [K, B, D]
    W1P = (f["W1"].astype(np.float32).reshape(CCH, 128, RH)
           .transpose(1, 0, 2))                     # [128, CCH, RH]
    WP = np.ascontiguousarray(
        f["W_enc"].astype(_BF).reshape(K, DCH, 128, H).transpose(0, 2, 1, 3))
    w4 = _softmax(f["fusion_w"].astype(np.float64).ravel()).astype(np.float32)
    b3p = (f["b3"].astype(np.float32) + f["prior"].astype(np.float32)).ravel()
    P = np.zeros((128, 8), dtype=np.float32)
    P[0:RH, 0] = f["b1"].astype(np.float32).ravel()
    P[0:RH, 1] = f["g_ln"].astype(np.float32).ravel()
    P[0:RH, 2] = f["beta_ln"].astype(np.float32).ravel()
    P[0:RH2, 3] = f["b2"].astype(np.float32).ravel()
    P[0:K, 4] = b3p
    Q = np.zeros((RH, RH2 + K), dtype=np.float32)
    Q[:, 0:RH2] = f["W2"].astype(np.float32)
    Q[0:RH2, RH2:RH2 + K] = f["W3"].astype(np.float32)
    shared = {
        "P": P,
        "Q": Q,
        "WP": WP,
        "b_encP": np.ascontiguousarray(f["b_enc"].astype(_BF)),
        "w4": w4.reshape(1, K),
    }
    in_maps = []
    for i in range(N_CORES):
        sl = slice(i * BL, (i + 1) * BL)
        m = dict(shared)
        # ctxP[p, c, b] = context[b, c*128+p]; packed with W1P -> CW
        ctxP = (f["context"][sl].astype(np.float32).T.reshape(CCH, 128, BL)
                .transpose(1, 0, 2))
        m["CW"] = np.ascontiguousarray(
            np.concatenate([ctxP, W1P], axis=2))
        # xP[k, p, c, b] = x[k, b, c*128+p]
        m["xP"] = np.ascontiguousarray(
            x_bf[:, sl, :].transpose(0, 2, 1).reshape(K, DCH, 128, BL)
            .transpose(0, 2, 1, 3))
        # gumP[p, t, k] = gumbel[t*128+p, k]
        m["gumP"] = np.ascontiguousarray(
            f["gumbel"][sl].astype(np.float32).reshape(NBT, 128, K)
            .transpose(1, 0, 2))
        in_maps.append(m)
    return in_maps


def kernel(**inputs):
    nc = _get_nc()
    in_maps = _make_in_maps(inputs)
    res = run_bass_kernel_spmd(nc, in_maps, core_ids=list(range(N_CORES)))
    return np.concatenate([res.results[i]["out"] for i in range(N_CORES)],
                          axis=0)
